# revision 28
# baseline (speedup 1.0000x reference)
"""Trainium2 Bass kernel for DGL DigitCapsuleLayer (capsule routing).

Reference computation (B=256, I=1152, J=10, D=16, K=8, 3 routing iters):
    u_hat[i,j,b,d] = sum_k W[i,j,d,k] * x[b,k,i]
    b_ij = 0
    for 3 iters:
        c = softmax(b_ij, axis=j)
        s[j,b,d] = sum_i c[i,j] * u_hat[i,j,b,d]
        v = squash(s, axis=d)
        b_ij += mean_b sum_d v[j,b,d] * u_hat[i,j,b,d]
    out = v transposed to (B, J, D, 1)

Strategy: data-parallel over batch B across 8 cores (B_loc=32), W replicated.
u_hat (189 MB full / 23.6 MB per core) is NEVER materialized; both per-
iteration contractions are factored through x and W:
    s[b,(j,d)]     = sum_(k,i) X2[(k,i),b] * (c[i,j] * W2[(k,i),(j,d)])
    Q[(k,i),(j,d)] = sum_b X3[b,(k,i)] * (v[b,(j,d)]/B)
    db[i,j]        = sum_(k,d) W2[(k,i),(j,d)] * Q[(k,i),(j,d)]
db is AllGather'd + locally summed across cores each iteration (cheaper than
AllReduce at this size). Matmul operands are fp16 (fp32 PE matmul is 4x
slower); accumulation fp32. sqrt comes from a DVE fast-inverse-sqrt
(bit trick + 2 Newton steps) so the ACT engine needs only one function
table for the whole kernel (table reloads cost ~1.3us each).
"""

import numpy as np

B, I, J, D, K = 256, 1152, 10, 16, 8
NCORES = 8
BL = B // NCORES            # 32 local batches
JD = J * D                  # 160
IK = I * K                  # 9216
NCH = IK // 128             # 72 chunks of the (k,i) contraction axis
G = I // 128                # 9 i-groups per k
RI = 3                      # routing iterations
BPG = 3                     # Q-matmul chunks packed per PSUM bank (3*160<=512)

_CACHE = {}


def _build_nc(skip_cc=False, skip_wdma=False, iters=RI, skip_agree=False,
              skip_spass=False):
    import concourse.bacc as bacc
    import concourse.mybir as mybir
    import concourse.tile as tile

    f32 = mybir.dt.float32
    f16 = mybir.dt.float16

    nc = bacc.Bacc(None, target_bir_lowering=False, debug=False)

    # DRAM images are laid out exactly as the SBUF tiles want them.
    x3d = nc.dram_tensor("x3", [BL, IK], f16, kind="ExternalInput")
    x2d = nc.dram_tensor("x2", [128, NCH * BL], f16, kind="ExternalInput")
    w2d = nc.dram_tensor("w2", [128, NCH * JD], f16, kind="ExternalInput")
    outd = nc.dram_tensor("out", [BL, JD], f32, kind="ExternalOutput")

    with tile.TileContext(nc) as tc:
        with (
            tc.tile_pool(name="big", bufs=1) as big,
            tc.tile_pool(name="small", bufs=1) as small,
            tc.tile_pool(name="scratch", bufs=2) as scratch,
            tc.tile_pool(name="spsum", bufs=2, space="PSUM") as spsum,
            tc.tile_pool(name="qpsum", bufs=6, space="PSUM") as qpsum,
            tc.tile_pool(name="dram", bufs=1, space="DRAM") as dram,
        ):
            w2s = big.tile([128, NCH, JD], f16, tag="w2s")
            w2c = big.tile([128, NCH, JD], f16, tag="w2c")
            pbuf = big.tile([128, NCH, JD], f16, tag="pbuf")
            x2s = big.tile([128, NCH, BL], f16, tag="x2s")
            # x3 is padded (2304+16 per quarter) so its DMA access pattern
            # keeps two free dims: a single merged free dim gets shredded
            # into hundreds of min-latency descriptors by the DMA splitter.
            x3s = big.tile([BL, 4, IK // 4 + 16], f16, tag="x3s")

            bb = small.tile([128, G, J], f32, tag="bb")         # running b_ij
            dpart = small.tile([128, G * J], f16, tag="dpart")  # local db
            if skip_agree:
                nc.vector.memset(dpart[:], 0.0)

            # ---- load inputs; x2 first (small, unblocks the s-pass), w2 in
            # chunks so early matmuls start before the full load; x3 is only
            # needed by the agreement pass and overlaps the first s-pass ----
            nc.sync.dma_start(x2s[:].rearrange("p c m -> p (c m)"), x2d[:])
            if not skip_wdma:
                w2flat = w2s[:].rearrange("p c m -> p (c m)")
                st = NCH * JD // 4
                for q in range(4):
                    nc.sync.dma_start(
                        w2flat[:, q * st:(q + 1) * st],
                        w2d[:, q * st:(q + 1) * st],
                    )
            else:
                nc.vector.memset(w2s[:].rearrange("p c m -> p (c m)"), 0.0)
            nc.sync.dma_start(
                x3s[:, :, 0:IK // 4],
                x3d[:].rearrange("b (u m) -> b u m", u=4),
            )

            def s_pass(wtile):
                """s[b,(j,d)] accumulated over all 72 (k,i) chunks."""
                s_ps = spsum.tile([BL, JD], f32, tag="s_ps")
                for c in range(NCH):
                    nc.tensor.matmul(
                        s_ps[:],
                        x2s[:, c, :],
                        wtile[:, c, :],
                        start=(c == 0),
                        stop=(c == NCH - 1),
                    )
                return s_ps

            def squash(s_ps, sigma, rho, out_dtype):
                """v_eff[b,(j,d)] = rho * squash(sigma * s).

                f = sigma*rho*sqrt(sq)/(1+sq) with sq = (sigma*s)^2 summed
                over d; sqrt comes from a DVE fast-rsqrt (no ACT table swap).
                """
                # ACT Square (present in every ACT table, so no table swap;
                # DVE cannot read two PSUM operands). q is the unscaled sum.
                sq2 = scratch.tile([BL, JD], f32, tag="sq2")
                nc.scalar.activation(
                    sq2[:], s_ps[:], mybir.ActivationFunctionType.Square,
                    bias=0.0, scale=1.0,
                )
                sq = scratch.tile([BL, J], f32, tag="sq")
                nc.vector.tensor_reduce(
                    sq[:], sq2[:].rearrange("b (j d) -> b j d", d=D),
                    axis=mybir.AxisListType.X, op=mybir.AluOpType.add,
                )
                # sq here is q = sum(s^2) (unscaled); true sq_t = sigma^2 * q:
                #   v_eff = s * rho*sigma^2*sqrt(q) / (1 + sigma^2*q)
                onep = scratch.tile([BL, J], f32, tag="onep")
                nc.vector.tensor_scalar(
                    onep[:], sq[:], float(sigma * sigma), 1.0,
                    op0=mybir.AluOpType.mult, op1=mybir.AluOpType.add,
                )
                rec = scratch.tile([BL, J], f32, tag="rec")
                nc.vector.reciprocal(rec[:], onep[:])
                # rsqrt(q) on DVE: bit-trick seed + 2 Newton steps (rel err
                # ~5e-6). Keeps Ln/Exp off ACT so one function table serves
                # the whole kernel (table reloads cost ~1.3us each).
                i32 = mybir.dt.int32
                yt = scratch.tile([BL, J], f32, tag="yt")
                t1 = scratch.tile([BL, J], i32, tag="t1")
                nc.vector.tensor_scalar(
                    t1[:], sq[:].bitcast(i32), 1, None,
                    op0=mybir.AluOpType.arith_shift_right,
                )
                nc.vector.tensor_scalar(
                    yt[:].bitcast(i32), t1[:], -1, 0x5F3759DF,
                    op0=mybir.AluOpType.mult, op1=mybir.AluOpType.add,
                )
                ya = scratch.tile([BL, J], f32, tag="ya")
                yb = scratch.tile([BL, J], f32, tag="yb")
                for _ in range(2):
                    nc.vector.tensor_mul(ya[:], yt[:], yt[:])
                    nc.vector.tensor_mul(yb[:], sq[:], ya[:])
                    nc.vector.tensor_scalar(
                        yb[:], yb[:], -0.5, 1.5,
                        op0=mybir.AluOpType.mult, op1=mybir.AluOpType.add,
                    )
                    nc.vector.tensor_mul(yt[:], yt[:], yb[:])
                # sqrt(q) = q * rsqrt(q);  f = (q*rho*sigma^2*y) * rec
                tq = scratch.tile([BL, J], f32, tag="tq")
                nc.vector.scalar_tensor_tensor(
                    tq[:], sq[:], float(rho * sigma * sigma), yt[:],
                    op0=mybir.AluOpType.mult, op1=mybir.AluOpType.mult,
                )
                f = scratch.tile([BL, J], f32, tag="f")
                nc.vector.tensor_mul(f[:], tq[:], rec[:])
                v = scratch.tile([BL, JD], out_dtype, tag="v" + str(out_dtype))
                nc.vector.tensor_tensor(
                    v[:].rearrange("b (j d) -> b j d", d=D),
                    s_ps[:].rearrange("b (j d) -> b j d", d=D),
                    f[:].unsqueeze(-1).broadcast_to((BL, J, D)),
                    mybir.AluOpType.mult,
                )
                return v

            def agreement(v16):
                """dpart[i_g, g*J+j] = sum_(k,d) W2*Q, Q = X3^T @ v16.

                Q chunks land 3-per-PSUM-bank. Evacuation alternates
                ACT Copy / DVE multiply-by-W2 straight from PSUM; the ACT
                half gets its W2 multiply as one strided 2x TT per half.
                The (k,d) sum runs as pairwise TT-add trees (TensorReduce
                is 1x on DVE; unit-stride fp16 adds are 2x).
                """
                ngrp = NCH // BPG            # 24 evac groups
                hg = ngrp // 2               # groups per half
                t8 = scratch.tile([128, NCH * J * (D // 2)], f16, tag="t8")
                with nc.allow_low_precision("16-term partial sums, db ~1e-3"):
                    for half in range(2):
                        for t in range(half * hg, (half + 1) * hg):
                            q_ps = qpsum.tile([128, BPG * JD], f32,
                                              tag="q_ps")
                            for u in range(BPG):
                                c = t * BPG + u
                                cpq = IK // 4 // 128  # chunks per quarter
                                nc.tensor.matmul(
                                    q_ps[:, u * JD:(u + 1) * JD],
                                    x3s[:, c // cpq,
                                        (c % cpq) * 128:(c % cpq + 1) * 128],
                                    v16[:],
                                    start=True,
                                    stop=True,
                                )
                            sl = pbuf[:, t * BPG:(t + 1) * BPG, :].rearrange(
                                "p c m -> p (c m)")
                            if t % 2 == 0:
                                # ACT evacuates even groups (W2 mul deferred)
                                nc.scalar.activation(
                                    sl, q_ps[:],
                                    mybir.ActivationFunctionType.Copy,
                                    bias=0.0, scale=1.0,
                                )
                            else:
                                # DVE muls odd groups by W2 right from PSUM
                                nc.vector.tensor_tensor(
                                    sl, q_ps[:],
                                    w2s[:, t * BPG:(t + 1) * BPG, :]
                                    .rearrange("p c m -> p (c m)"),
                                    mybir.AluOpType.mult,
                                )
                        # W2 multiply for this half's ACT (even) groups
                        lo, hi_ = half * hg, (half + 1) * hg
                        ev = pbuf[:, lo * BPG:hi_ * BPG, :].rearrange(
                            "p (t two u) m -> p t two (u m)", two=2, u=BPG
                        )[:, :, 0, :]
                        evw = w2s[:, lo * BPG:hi_ * BPG, :].rearrange(
                            "p (t two u) m -> p t two (u m)", two=2, u=BPG
                        )[:, :, 0, :]
                        nc.vector.tensor_tensor(
                            ev, ev, evw, mybir.AluOpType.mult)
                        # first d-tree level for this half (d16 -> d8)
                        nh = hg * BPG * J    # 360 (j-groups in half)
                        src = pbuf[:, lo * BPG:hi_ * BPG, :].rearrange(
                            "p c (n w) -> p (c n) w", w=D)
                        dst = t8[:, half * nh * 8:(half + 1) * nh * 8]
                        nc.vector.tensor_tensor(
                            dst.rearrange("p (n w) -> p n w", w=D // 2),
                            src[:, :, 0:D // 2], src[:, :, D // 2:D],
                            mybir.AluOpType.add)
                    # remaining d levels (8 -> 1), then k levels (8 -> 1)
                    flat = t8[:]
                    width = D // 2
                    while width > 1:
                        half_w = width // 2
                        nxt = scratch.tile(
                            [128, NCH * J * half_w], f16,
                            tag="dtree%d" % half_w)
                        v = flat.rearrange("p (n w) -> p n w", w=width)
                        nc.vector.tensor_tensor(
                            nxt[:].rearrange("p (n w) -> p n w", w=half_w),
                            v[:, :, 0:half_w], v[:, :, half_w:width],
                            mybir.AluOpType.add)
                        cur = nxt
                        flat = nxt[:]
                        width = half_w
                    span = NCH * J  # 720 = (k=8) * 90
                    while span > G * J:
                        half_s = span // 2
                        nxt = (dpart if half_s == G * J else
                               scratch.tile([128, half_s], f16,
                                            tag="ktree%d" % half_s))
                        nc.vector.tensor_tensor(
                            nxt[:], cur[:, 0:half_s], cur[:, half_s:span],
                            mybir.AluOpType.add)
                        cur = nxt
                        span = half_s

            def softmax_cexp():
                """c = softmax(bb over j), expanded along d as fp16."""
                e = scratch.tile([128, G, J], f32, tag="e")
                nc.scalar.activation(
                    e[:], bb[:], mybir.ActivationFunctionType.Exp,
                    bias=0.0, scale=1.0,
                )
                den = scratch.tile([128, G], f32, tag="den")
                nc.vector.tensor_reduce(
                    den[:], e[:], axis=mybir.AxisListType.X,
                    op=mybir.AluOpType.add,
                )
                rden = scratch.tile([128, G], f32, tag="rden")
                nc.vector.reciprocal(rden[:], den[:])
                cc = scratch.tile([128, G, J], f16, tag="cc")
                nc.vector.tensor_tensor(
                    cc[:], e[:],
                    rden[:].unsqueeze(-1).broadcast_to((128, G, J)),
                    mybir.AluOpType.mult,
                )
                # expand c along d once (small 1x copy) so the per-k fold
                # multiplies are unit-stride on every operand (2x DVE mode)
                cexp = scratch.tile([128, G, J, D], f16, tag="cexp")
                nc.vector.tensor_copy(
                    cexp[:],
                    cc[:].unsqueeze(-1).broadcast_to((128, G, J, D)),
                )
                return cexp

            def s_pass_folded(cexp):
                """Fold w2c = c*w2 one k-slice at a time, with the s-pass
                matmuls for that slice issued right behind the fold."""
                s_ps = spsum.tile([BL, JD], f32, tag="s_ps")
                cv = cexp[:].rearrange("p g j d -> p (g j d)")
                wck = w2c[:].rearrange("p (k g) m -> p k (g m)", k=K)
                wsk = w2s[:].rearrange("p (k g) m -> p k (g m)", k=K)
                for k in range(K):
                    nc.vector.tensor_tensor(
                        wck[:, k, :], wsk[:, k, :], cv,
                        mybir.AluOpType.mult,
                    )
                    for c in range(k * G, (k + 1) * G):
                        nc.tensor.matmul(
                            s_ps[:],
                            x2s[:, c, :],
                            w2c[:, c, :],
                            start=(c == 0),
                            stop=(c == NCH - 1),
                        )
                return s_ps

            def do_ar(idx, dst_add):
                """AllGather per-core db and sum the 8 shards locally."""
                cc_in = dram.tile([128, G * J], f16, tag="cci%d" % idx)
                nc.sync.dma_start(cc_in[:], dpart[:])
                if not skip_cc:
                    cc_out = dram.tile([NCORES * 128, G * J], f16,
                                       tag="cco%d" % idx, addr_space="Shared")
                    nc.gpsimd.collective_compute(
                        "AllGather", mybir.AluOpType.bypass,
                        replica_groups=[list(range(NCORES))],
                        ins=[cc_in.opt()], outs=[cc_out.opt()],
                    )
                    gath = small.tile([128, NCORES, G * J], f16,
                                      tag="gath%d" % idx)
                    # DRAM (r, p, gj) -> SBUF (p, r, gj)
                    nc.sync.dma_start(
                        gath[:],
                        cc_out[:].rearrange("(r p) m -> p r m", r=NCORES),
                    )
                    src = gath[:].rearrange("p r m -> p m r")
                else:
                    gath = small.tile([128, G * J], f16, tag="gath%d" % idx)
                    nc.sync.dma_start(gath[:], cc_in[:])
                    src = gath[:].unsqueeze(-1)

                tot = small.tile([128, G * J], f32, tag="tot%d" % idx)
                nc.vector.tensor_reduce(
                    tot[:], src, axis=mybir.AxisListType.X,
                    op=mybir.AluOpType.add,
                )
                if not dst_add:
                    nc.vector.tensor_copy(
                        bb[:].rearrange("p g j -> p (g j)"), tot[:])
                else:
                    nc.vector.tensor_add(
                        bb[:].rearrange("p g j -> p (g j)"),
                        bb[:].rearrange("p g j -> p (g j)"),
                        tot[:],
                    )

            # =========== iteration 1 (c uniform = 1/J) ===========
            s_ps = s_pass(w2s)
            v16 = squash(s_ps, 1.0 / J, 1.0 / B, f16)
            if not skip_agree:
                agreement(v16)
            if iters >= 2:
                do_ar(0, dst_add=False)

                # =========== iteration 2 ===========
                cexp = softmax_cexp()
                if not skip_spass:
                    s_ps = s_pass_folded(cexp)
                v16 = squash(s_ps, 1.0, 1.0 / B, f16)
                if not skip_agree:
                    agreement(v16)
            if iters >= 3:
                do_ar(1, dst_add=True)

                # =========== iteration 3 (output) ===========
                cexp = softmax_cexp()
                if not skip_spass:
                    s_ps = s_pass_folded(cexp)
            vout = squash(s_ps, 1.0, 1.0, f32)
            nc.sync.dma_start(outd[:], vout[:])

    nc.compile()
    return nc


def _get_nc(**kw):
    key = tuple(sorted(kw.items()))
    if key not in _CACHE:
        _CACHE[key] = _build_nc(**kw)
    return _CACHE[key]


def _make_in_maps(x, W):
    # W2[(k,i),(j,d)] = W[i,j,d,k], chunked to the SBUF partition image
    w2 = np.ascontiguousarray(W.transpose(3, 0, 1, 2)).reshape(IK, JD)
    w2img = np.ascontiguousarray(
        w2.reshape(NCH, 128, JD).transpose(1, 0, 2)
    ).reshape(128, NCH * JD).astype(np.float16)
    in_maps = []
    for c in range(NCORES):
        xl = x[c * BL:(c + 1) * BL]          # (BL, K, I) float32
        x3 = xl.reshape(BL, IK).astype(np.float16)
        x2 = np.ascontiguousarray(xl.reshape(BL, IK).T)  # (IK, BL)
        x2img = np.ascontiguousarray(
            x2.reshape(NCH, 128, BL).transpose(1, 0, 2)
        ).reshape(128, NCH * BL).astype(np.float16)
        in_maps.append({"x3": x3, "x2": x2img, "w2": w2img})
    return in_maps


def kernel(x, W):
    from concourse.bass_utils import run_bass_kernel_spmd

    x = np.asarray(x, dtype=np.float32)
    W = np.asarray(W, dtype=np.float32)
    nc = _get_nc()
    in_maps = _make_in_maps(x, W)
    res = run_bass_kernel_spmd(nc, in_maps, core_ids=list(range(NCORES)))
    parts = [res.results[c]["out"].reshape(BL, J, D) for c in range(NCORES)]
    return np.concatenate(parts, axis=0)[..., None].astype(np.float32)


# revision 29
# speedup vs baseline: 1.0090x; 1.0090x over previous
"""Trainium2 Bass kernel for DGL DigitCapsuleLayer (capsule routing).

Reference computation (B=256, I=1152, J=10, D=16, K=8, 3 routing iters):
    u_hat[i,j,b,d] = sum_k W[i,j,d,k] * x[b,k,i]
    b_ij = 0
    for 3 iters:
        c = softmax(b_ij, axis=j)
        s[j,b,d] = sum_i c[i,j] * u_hat[i,j,b,d]
        v = squash(s, axis=d)
        b_ij += mean_b sum_d v[j,b,d] * u_hat[i,j,b,d]
    out = v transposed to (B, J, D, 1)

Strategy: data-parallel over batch B across 8 cores (B_loc=32), W replicated.
u_hat (189 MB full / 23.6 MB per core) is NEVER materialized; both per-
iteration contractions are factored through x and W:
    s[b,(j,d)]     = sum_(k,i) X2[(k,i),b] * (c[i,j] * W2[(k,i),(j,d)])
    Q[(k,i),(j,d)] = sum_b X3[b,(k,i)] * (v[b,(j,d)]/B)
    db[i,j]        = sum_(k,d) W2[(k,i),(j,d)] * Q[(k,i),(j,d)]
db is AllGather'd + locally summed across cores each iteration (cheaper than
AllReduce at this size). Matmul operands are fp16 (fp32 PE matmul is 4x
slower); accumulation fp32. sqrt comes from a DVE fast-inverse-sqrt
(bit trick + 2 Newton steps) so the ACT engine needs only one function
table for the whole kernel (table reloads cost ~1.3us each).
"""

import numpy as np

B, I, J, D, K = 256, 1152, 10, 16, 8
NCORES = 8
BL = B // NCORES            # 32 local batches
JD = J * D                  # 160
IK = I * K                  # 9216
NCH = IK // 128             # 72 chunks of the (k,i) contraction axis
G = I // 128                # 9 i-groups per k
RI = 3                      # routing iterations
BPG = 3                     # Q-matmul chunks packed per PSUM bank (3*160<=512)

_CACHE = {}


def _build_nc(skip_cc=False, skip_wdma=False, iters=RI, skip_agree=False,
              skip_spass=False):
    import concourse.bacc as bacc
    import concourse.mybir as mybir
    import concourse.tile as tile

    f32 = mybir.dt.float32
    f16 = mybir.dt.float16

    nc = bacc.Bacc(None, target_bir_lowering=False, debug=False)

    # DRAM images are laid out exactly as the SBUF tiles want them.
    x3d = nc.dram_tensor("x3", [BL, IK], f16, kind="ExternalInput")
    x2d = nc.dram_tensor("x2", [128, NCH * BL], f16, kind="ExternalInput")
    w2d = nc.dram_tensor("w2", [128, NCH * JD], f16, kind="ExternalInput")
    outd = nc.dram_tensor("out", [BL, JD], f32, kind="ExternalOutput")

    with tile.TileContext(nc) as tc:
        with (
            tc.tile_pool(name="big", bufs=1) as big,
            tc.tile_pool(name="small", bufs=1) as small,
            tc.tile_pool(name="scratch", bufs=2) as scratch,
            tc.tile_pool(name="spsum", bufs=2, space="PSUM") as spsum,
            tc.tile_pool(name="qpsum", bufs=6, space="PSUM") as qpsum,
            tc.tile_pool(name="dram", bufs=1, space="DRAM") as dram,
        ):
            w2s = big.tile([128, NCH, JD], f16, tag="w2s")
            w2c = big.tile([128, NCH, JD], f16, tag="w2c")
            pbuf = big.tile([128, NCH, JD], f16, tag="pbuf")
            x2s = big.tile([128, NCH, BL], f16, tag="x2s")
            # x3 is padded (2304+16 per quarter) so its DMA access pattern
            # keeps two free dims: a single merged free dim gets shredded
            # into hundreds of min-latency descriptors by the DMA splitter.
            x3s = big.tile([BL, 4, IK // 4 + 16], f16, tag="x3s")

            bb = small.tile([128, G, J], f32, tag="bb")         # running b_ij
            dpart = small.tile([128, G * J], f16, tag="dpart")  # local db
            if skip_agree:
                nc.vector.memset(dpart[:], 0.0)

            # ---- load inputs; x2 first (small, unblocks the s-pass), w2 in
            # chunks so early matmuls start before the full load; x3 is only
            # needed by the agreement pass and overlaps the first s-pass ----
            nc.sync.dma_start(x2s[:].rearrange("p c m -> p (c m)"), x2d[:])
            if not skip_wdma:
                w2flat = w2s[:].rearrange("p c m -> p (c m)")
                st = NCH * JD // 4
                for q in range(4):
                    nc.sync.dma_start(
                        w2flat[:, q * st:(q + 1) * st],
                        w2d[:, q * st:(q + 1) * st],
                    )
            else:
                nc.vector.memset(w2s[:].rearrange("p c m -> p (c m)"), 0.0)
            nc.sync.dma_start(
                x3s[:, :, 0:IK // 4],
                x3d[:].rearrange("b (u m) -> b u m", u=4),
            )

            def s_pass(wtile):
                """s[b,(j,d)] accumulated over all 72 (k,i) chunks."""
                s_ps = spsum.tile([BL, JD], f32, tag="s_ps")
                for c in range(NCH):
                    nc.tensor.matmul(
                        s_ps[:],
                        x2s[:, c, :],
                        wtile[:, c, :],
                        start=(c == 0),
                        stop=(c == NCH - 1),
                    )
                return s_ps

            def squash(s_ps, sigma, rho, out_dtype):
                """v_eff[b,(j,d)] = rho * squash(sigma * s).

                f = sigma*rho*sqrt(sq)/(1+sq) with sq = (sigma*s)^2 summed
                over d; sqrt comes from a DVE fast-rsqrt (no ACT table swap).
                """
                # ACT Square (present in every ACT table, so no table swap;
                # DVE cannot read two PSUM operands). q is the unscaled sum.
                sq2 = scratch.tile([BL, JD], f32, tag="sq2")
                nc.scalar.activation(
                    sq2[:], s_ps[:], mybir.ActivationFunctionType.Square,
                    bias=0.0, scale=1.0,
                )
                sq = scratch.tile([BL, J], f32, tag="sq")
                nc.vector.tensor_reduce(
                    sq[:], sq2[:].rearrange("b (j d) -> b j d", d=D),
                    axis=mybir.AxisListType.X, op=mybir.AluOpType.add,
                )
                # sq here is q = sum(s^2) (unscaled); true sq_t = sigma^2 * q:
                #   v_eff = s * rho*sigma^2*sqrt(q) / (1 + sigma^2*q)
                onep = scratch.tile([BL, J], f32, tag="onep")
                nc.vector.tensor_scalar(
                    onep[:], sq[:], float(sigma * sigma), 1.0,
                    op0=mybir.AluOpType.mult, op1=mybir.AluOpType.add,
                )
                rec = scratch.tile([BL, J], f32, tag="rec")
                nc.vector.reciprocal(rec[:], onep[:])
                # rsqrt(q) on DVE: bit-trick seed + 2 Newton steps (rel err
                # ~5e-6). Keeps Ln/Exp off ACT so one function table serves
                # the whole kernel (table reloads cost ~1.3us each).
                i32 = mybir.dt.int32
                yt = scratch.tile([BL, J], f32, tag="yt")
                t1 = scratch.tile([BL, J], i32, tag="t1")
                nc.vector.tensor_scalar(
                    t1[:], sq[:].bitcast(i32), 1, None,
                    op0=mybir.AluOpType.arith_shift_right,
                )
                nc.vector.tensor_scalar(
                    yt[:].bitcast(i32), t1[:], -1, 0x5F3759DF,
                    op0=mybir.AluOpType.mult, op1=mybir.AluOpType.add,
                )
                ya = scratch.tile([BL, J], f32, tag="ya")
                yb = scratch.tile([BL, J], f32, tag="yb")
                for _ in range(2):
                    nc.vector.tensor_mul(ya[:], yt[:], yt[:])
                    nc.vector.tensor_mul(yb[:], sq[:], ya[:])
                    nc.vector.tensor_scalar(
                        yb[:], yb[:], -0.5, 1.5,
                        op0=mybir.AluOpType.mult, op1=mybir.AluOpType.add,
                    )
                    nc.vector.tensor_mul(yt[:], yt[:], yb[:])
                # sqrt(q) = q * rsqrt(q);  f = (q*rho*sigma^2*y) * rec
                tq = scratch.tile([BL, J], f32, tag="tq")
                nc.vector.scalar_tensor_tensor(
                    tq[:], sq[:], float(rho * sigma * sigma), yt[:],
                    op0=mybir.AluOpType.mult, op1=mybir.AluOpType.mult,
                )
                f = scratch.tile([BL, J], f32, tag="f")
                nc.vector.tensor_mul(f[:], tq[:], rec[:])
                v = scratch.tile([BL, JD], out_dtype, tag="v" + str(out_dtype))
                nc.vector.tensor_tensor(
                    v[:].rearrange("b (j d) -> b j d", d=D),
                    s_ps[:].rearrange("b (j d) -> b j d", d=D),
                    f[:].unsqueeze(-1).broadcast_to((BL, J, D)),
                    mybir.AluOpType.mult,
                )
                return v

            def agreement(v16):
                """dpart[i_g, g*J+j] = sum_(k,d) W2*Q, Q = X3^T @ v16.

                Q chunks land 3-per-PSUM-bank. Evacuation alternates
                ACT Copy / DVE multiply-by-W2 straight from PSUM; the ACT
                half gets its W2 multiply as one strided 2x TT per half.
                The (k,d) sum runs as pairwise TT-add trees (TensorReduce
                is 1x on DVE; unit-stride fp16 adds are 2x).
                """
                ngrp = NCH // BPG            # 24 evac groups
                hg = ngrp // 2               # groups per half
                t8 = scratch.tile([128, NCH * J * (D // 2)], f16, tag="t8")
                with nc.allow_low_precision("16-term partial sums, db ~1e-3"):
                    for half in range(2):
                        for t in range(half * hg, (half + 1) * hg):
                            q_ps = qpsum.tile([128, BPG * JD], f32,
                                              tag="q_ps")
                            for u in range(BPG):
                                c = t * BPG + u
                                cpq = IK // 4 // 128  # chunks per quarter
                                nc.tensor.matmul(
                                    q_ps[:, u * JD:(u + 1) * JD],
                                    x3s[:, c // cpq,
                                        (c % cpq) * 128:(c % cpq + 1) * 128],
                                    v16[:],
                                    start=True,
                                    stop=True,
                                )
                            sl = pbuf[:, t * BPG:(t + 1) * BPG, :].rearrange(
                                "p c m -> p (c m)")
                            if t % 3 != 2:
                                # ACT evacuates 2 of 3 groups (W2 deferred)
                                nc.scalar.activation(
                                    sl, q_ps[:],
                                    mybir.ActivationFunctionType.Copy,
                                    bias=0.0, scale=1.0,
                                )
                            else:
                                # DVE muls every 3rd group right from PSUM
                                nc.vector.tensor_tensor(
                                    sl, q_ps[:],
                                    w2s[:, t * BPG:(t + 1) * BPG, :]
                                    .rearrange("p c m -> p (c m)"),
                                    mybir.AluOpType.mult,
                                )
                        # W2 multiply for this half's ACT groups: two
                        # strided runs (t%3==0 and t%3==1), 2x DVE mode
                        lo, hi_ = half * hg, (half + 1) * hg
                        for ph in range(2):
                            ev = pbuf[:, lo * BPG:hi_ * BPG, :].rearrange(
                                "p (t three u) m -> p t three (u m)",
                                three=3, u=BPG)[:, :, ph, :]
                            evw = w2s[:, lo * BPG:hi_ * BPG, :].rearrange(
                                "p (t three u) m -> p t three (u m)",
                                three=3, u=BPG)[:, :, ph, :]
                            nc.vector.tensor_tensor(
                                ev, ev, evw, mybir.AluOpType.mult)
                        # first d-tree level for this half (d16 -> d8)
                        nh = hg * BPG * J    # 360 (j-groups in half)
                        src = pbuf[:, lo * BPG:hi_ * BPG, :].rearrange(
                            "p c (n w) -> p (c n) w", w=D)
                        dst = t8[:, half * nh * 8:(half + 1) * nh * 8]
                        nc.vector.tensor_tensor(
                            dst.rearrange("p (n w) -> p n w", w=D // 2),
                            src[:, :, 0:D // 2], src[:, :, D // 2:D],
                            mybir.AluOpType.add)
                    # remaining d levels (8 -> 1), then k levels (8 -> 1)
                    flat = t8[:]
                    width = D // 2
                    while width > 1:
                        half_w = width // 2
                        nxt = scratch.tile(
                            [128, NCH * J * half_w], f16,
                            tag="dtree%d" % half_w)
                        v = flat.rearrange("p (n w) -> p n w", w=width)
                        nc.vector.tensor_tensor(
                            nxt[:].rearrange("p (n w) -> p n w", w=half_w),
                            v[:, :, 0:half_w], v[:, :, half_w:width],
                            mybir.AluOpType.add)
                        cur = nxt
                        flat = nxt[:]
                        width = half_w
                    span = NCH * J  # 720 = (k=8) * 90
                    while span > G * J:
                        half_s = span // 2
                        nxt = (dpart if half_s == G * J else
                               scratch.tile([128, half_s], f16,
                                            tag="ktree%d" % half_s))
                        nc.vector.tensor_tensor(
                            nxt[:], cur[:, 0:half_s], cur[:, half_s:span],
                            mybir.AluOpType.add)
                        cur = nxt
                        span = half_s

            def softmax_cexp():
                """c = softmax(bb over j), expanded along d as fp16."""
                e = scratch.tile([128, G, J], f32, tag="e")
                nc.scalar.activation(
                    e[:], bb[:], mybir.ActivationFunctionType.Exp,
                    bias=0.0, scale=1.0,
                )
                den = scratch.tile([128, G], f32, tag="den")
                nc.vector.tensor_reduce(
                    den[:], e[:], axis=mybir.AxisListType.X,
                    op=mybir.AluOpType.add,
                )
                rden = scratch.tile([128, G], f32, tag="rden")
                nc.vector.reciprocal(rden[:], den[:])
                cc = scratch.tile([128, G, J], f16, tag="cc")
                nc.vector.tensor_tensor(
                    cc[:], e[:],
                    rden[:].unsqueeze(-1).broadcast_to((128, G, J)),
                    mybir.AluOpType.mult,
                )
                # expand c along d once (small 1x copy) so the per-k fold
                # multiplies are unit-stride on every operand (2x DVE mode)
                cexp = scratch.tile([128, G, J, D], f16, tag="cexp")
                nc.vector.tensor_copy(
                    cexp[:],
                    cc[:].unsqueeze(-1).broadcast_to((128, G, J, D)),
                )
                return cexp

            def s_pass_folded(cexp):
                """Fold w2c = c*w2 one k-slice at a time, with the s-pass
                matmuls for that slice issued right behind the fold."""
                s_ps = spsum.tile([BL, JD], f32, tag="s_ps")
                cv = cexp[:].rearrange("p g j d -> p (g j d)")
                wck = w2c[:].rearrange("p (k g) m -> p k (g m)", k=K)
                wsk = w2s[:].rearrange("p (k g) m -> p k (g m)", k=K)
                for k in range(K):
                    nc.vector.tensor_tensor(
                        wck[:, k, :], wsk[:, k, :], cv,
                        mybir.AluOpType.mult,
                    )
                    for c in range(k * G, (k + 1) * G):
                        nc.tensor.matmul(
                            s_ps[:],
                            x2s[:, c, :],
                            w2c[:, c, :],
                            start=(c == 0),
                            stop=(c == NCH - 1),
                        )
                return s_ps

            def do_ar(idx, dst_add):
                """AllGather per-core db and sum the 8 shards locally."""
                cc_in = dram.tile([128, G * J], f16, tag="cci%d" % idx)
                nc.sync.dma_start(cc_in[:], dpart[:])
                if not skip_cc:
                    cc_out = dram.tile([NCORES * 128, G * J], f16,
                                       tag="cco%d" % idx, addr_space="Shared")
                    nc.gpsimd.collective_compute(
                        "AllGather", mybir.AluOpType.bypass,
                        replica_groups=[list(range(NCORES))],
                        ins=[cc_in.opt()], outs=[cc_out.opt()],
                    )
                    gath = small.tile([128, NCORES, G * J], f16,
                                      tag="gath%d" % idx)
                    # DRAM (r, p, gj) -> SBUF (p, r, gj)
                    nc.sync.dma_start(
                        gath[:],
                        cc_out[:].rearrange("(r p) m -> p r m", r=NCORES),
                    )
                    src = gath[:].rearrange("p r m -> p m r")
                else:
                    gath = small.tile([128, G * J], f16, tag="gath%d" % idx)
                    nc.sync.dma_start(gath[:], cc_in[:])
                    src = gath[:].unsqueeze(-1)

                if not dst_add:
                    nc.vector.tensor_reduce(
                        bb[:].rearrange("p g j -> p (g j)"), src,
                        axis=mybir.AxisListType.X, op=mybir.AluOpType.add,
                    )
                else:
                    tot = small.tile([128, G * J], f32, tag="tot%d" % idx)
                    nc.vector.tensor_reduce(
                        tot[:], src, axis=mybir.AxisListType.X,
                        op=mybir.AluOpType.add,
                    )
                    nc.vector.tensor_add(
                        bb[:].rearrange("p g j -> p (g j)"),
                        bb[:].rearrange("p g j -> p (g j)"),
                        tot[:],
                    )

            # =========== iteration 1 (c uniform = 1/J) ===========
            s_ps = s_pass(w2s)
            v16 = squash(s_ps, 1.0 / J, 1.0 / B, f16)
            if not skip_agree:
                agreement(v16)
            if iters >= 2:
                do_ar(0, dst_add=False)

                # =========== iteration 2 ===========
                cexp = softmax_cexp()
                if not skip_spass:
                    s_ps = s_pass_folded(cexp)
                v16 = squash(s_ps, 1.0, 1.0 / B, f16)
                if not skip_agree:
                    agreement(v16)
            if iters >= 3:
                do_ar(1, dst_add=True)

                # =========== iteration 3 (output) ===========
                cexp = softmax_cexp()
                if not skip_spass:
                    s_ps = s_pass_folded(cexp)
            vout = squash(s_ps, 1.0, 1.0, f32)
            nc.sync.dma_start(outd[:], vout[:])

    nc.compile()
    return nc


def _get_nc(**kw):
    key = tuple(sorted(kw.items()))
    if key not in _CACHE:
        _CACHE[key] = _build_nc(**kw)
    return _CACHE[key]


def _make_in_maps(x, W):
    # W2[(k,i),(j,d)] = W[i,j,d,k], chunked to the SBUF partition image
    w2 = np.ascontiguousarray(W.transpose(3, 0, 1, 2)).reshape(IK, JD)
    w2img = np.ascontiguousarray(
        w2.reshape(NCH, 128, JD).transpose(1, 0, 2)
    ).reshape(128, NCH * JD).astype(np.float16)
    in_maps = []
    for c in range(NCORES):
        xl = x[c * BL:(c + 1) * BL]          # (BL, K, I) float32
        x3 = xl.reshape(BL, IK).astype(np.float16)
        x2 = np.ascontiguousarray(xl.reshape(BL, IK).T)  # (IK, BL)
        x2img = np.ascontiguousarray(
            x2.reshape(NCH, 128, BL).transpose(1, 0, 2)
        ).reshape(128, NCH * BL).astype(np.float16)
        in_maps.append({"x3": x3, "x2": x2img, "w2": w2img})
    return in_maps


def kernel(x, W):
    from concourse.bass_utils import run_bass_kernel_spmd

    x = np.asarray(x, dtype=np.float32)
    W = np.asarray(W, dtype=np.float32)
    nc = _get_nc()
    in_maps = _make_in_maps(x, W)
    res = run_bass_kernel_spmd(nc, in_maps, core_ids=list(range(NCORES)))
    parts = [res.results[c]["out"].reshape(BL, J, D) for c in range(NCORES)]
    return np.concatenate(parts, axis=0)[..., None].astype(np.float32)
